# revision 15
# baseline (speedup 1.0000x reference)
"""Contrastive loss on Trainium2 (8 NeuronCores, SPMD, Bass/Tile).

Math
----
reference:
    norms[i,j] = ||x_i||^2 + ||x_j||^2 - 2 x_i.x_j
    pos = sum((eq - I) * norms) / cnt_pos          eq[i,j] = [y_i == y_j]
    neg = sum((1 - eq) * relu(1 - norms)) / cnt_neg
    loss = (pos + neg) / 2

pos is computed exactly on the host via the class-sum identity (O(N*D)).
The device computes only the masked negative sum.

Device trick (fp8 DoubleRow, ONE matmul per output tile):
    u[i,j] = norms[i,j] - 1 + BIG * eq[i,j]          (BIG = 64 >= 1 + fp8 noise)

A single K=256 fp8 DoubleRow matmul packs both halves:
  - K-half 0 (128 rows): lhsT = -2 x_i^T, rhs = x_j^T          -> -2 G
  - K-half 1 (47 rows + zero pad): lhsT = [onehot; 1; sm1 hi/lo],
    rhs = [BIG*onehot; sq hi/lo; 1]                            -> BIG*eq + sq_j + (sq_i - 1)

Masked sums come out of u with ONE fused instruction per tile:
    ACT:     sum relu(-u)  = +sum_{eq=0} relu(1-norms)   (accum_out)
    DVE/GP:  sum min(u,0)  = -sum_{eq=0} relu(1-norms)   (accum_out)
eq pairs (incl. diagonal) land at u ~ d2-1+64 > 0 -> contribute 0.
fp8 margins (verified on data): min off-diag d2 ~ 121 >> 1, max value 205 < 240.

Work halving (symmetry): with 128-row blocks r and 128-col blocks c (64 of
each), let d = (c - r) mod 64. Summing blocks d=0 (weight 1), d=1..31
(weight 2), d=32 (weight 1) covers every ordered pair exactly once. Each
row-block processes a contiguous circular span of 33*128 = 4224 columns.

Sharding: core k owns global rows [1024k, 1024(k+1)). Its 8 row-blocks need
the circular column window [1024k, 1024k + 5120) — the host ships that
window per-core ("rolled" columns), so the device program is identical on
every core (pure SPMD). Per-core outputs are per-partition partial sums;
the host applies unit weights / counts and reduces (O(N) work).
"""

import numpy as np
from contextlib import ExitStack

import concourse.bass as bass
import concourse.bacc as bacc
import concourse.tile as tile
from concourse import mybir
import concourse.bass_utils as _bu
from concourse.bass_utils import run_bass_kernel_spmd

# walrus disables the LDWEIGHTS dedup pass by default; our inner loop issues
# 9 same-weight matmuls per row-block, so redundant LDW streams cost ~12us.
LDW_OPT = False   # walrus: "InstLdweights is not compatible with LDW optimization"
if LDW_OPT and not getattr(_bu, "_ldw_patch", False):
    _orig_run_command = _bu.run_command

    def _run_command_ldw(cmd, *a, **kw):
        cmd = ["--enable-ldw-opt=true" if c == "--enable-ldw-opt=false" else c
               for c in cmd]
        return _orig_run_command(cmd, *a, **kw)

    _bu.run_command = _run_command_ldw
    _bu._ldw_patch = True

N, D, C = 8192, 128, 43
MARGIN = 1.0
BIG = 64.0
P = 128
NCORES = 8
ROWS_PER_CORE = N // NCORES           # 1024
RB = ROWS_PER_CORE // P               # 8 row-blocks per core
LOCAL_COLS = ROWS_PER_CORE + 32 * P   # 5120: own rows + 32 blocks ahead
AUGK = C + 4                          # 47 aug rows (onehot + sq hi/lo + ones)
WARMUP = 110                          # PE warm-up matmuls during DMA wait
GP_CONSUME = False                    # gpsimd cannot read PSUM on TRN2

# Per row-block jj (local col base b = 128*jj) the 4224-col span splits as:
#   S : [b, b+128) + [b+4096, b+4224)   d0 + d32 blocks, weight 1
#   M0: [b+128, b+1152)    weight 2     (ACT)
#   M1: [b+1152, b+2176)   weight 2     (DVE)
#   M2: [b+2176, b+3200)   weight 2     (ACT)
#   M3: [b+3200, b+4096)   weight 2     (DVE, 896 cols)
UNITS_PER_JJ = 5
NPART = UNITS_PER_JJ * RB


def _unit_info():
    """Per unit: (weight, sign). sign=+1 for ACT relu(-u), -1 for min(u,0)."""
    w = np.zeros(NPART)
    s = np.zeros(NPART)
    for jj in range(RB):
        u = UNITS_PER_JJ * jj
        w[u + 0], s[u + 0] = 2.0, +1.0   # M0 (ACT)
        w[u + 1], s[u + 1] = 2.0, -1.0   # M1 (DVE)
        w[u + 2], s[u + 2] = 2.0, +1.0   # M2 (ACT)
        w[u + 3], s[u + 3] = 2.0, -1.0   # M3 (DVE)
        w[u + 4], s[u + 4] = 1.0, -1.0   # S d0+d32 (DVE)
    return w, s


UNIT_W, UNIT_SIGN = _unit_info()

_cache = {}
TRACE = False


def _build_bass():
    f32 = mybir.dt.float32
    fp8 = mybir.dt.float8e4
    nc = bacc.Bacc("TRN2", target_bir_lowering=False, debug=False)

    rx_d = nc.dram_tensor("rx", [P, 2, LOCAL_COLS], fp8, kind="ExternalInput").ap()
    wt_d = nc.dram_tensor("wt", [P, 2, ROWS_PER_CORE], fp8, kind="ExternalInput").ap()
    neg_out = nc.dram_tensor("neg_out", [P, NPART], f32, kind="ExternalOutput").ap()

    relu = mybir.ActivationFunctionType.Relu
    alu_min = mybir.AluOpType.min
    alu_add = mybir.AluOpType.add
    DR = mybir.MatmulPerfMode.DoubleRow

    with tile.TileContext(nc) as tc:
        with ExitStack() as ctx:
            const = ctx.enter_context(tc.tile_pool(name="const", bufs=1))
            psum = ctx.enter_context(tc.tile_pool(name="psum", bufs=3, space="PSUM"))
            psum_s = ctx.enter_context(tc.tile_pool(name="psum_s", bufs=2, space="PSUM"))
            scr_a = ctx.enter_context(tc.tile_pool(name="scr_a", bufs=2))
            scr_v = ctx.enter_context(tc.tile_pool(name="scr_v", bufs=2))
            scr_g = ctx.enter_context(tc.tile_pool(name="scr_g", bufs=2))

            # ---- constants / weights ----
            wu = const.tile([P, 256], fp8)         # warm-up weights/rhs
            nc.vector.memset(wu, 0.0)
            zbias = const.tile([P, 1], f32)
            nc.vector.memset(zbias, 0.0)
            negp = const.tile([P, NPART], f32)

            rxt = const.tile([P, 2, LOCAL_COLS], fp8)
            wt = const.tile([P, 2, ROWS_PER_CORE], fp8)

            # ---- input DMAs, split across both HWDGE queues, in need order.
            # aug half ships all 128 rows (rows 47.. are zeros baked in DRAM:
            # they meet zero weights, but garbage NaN would poison 0*NaN).
            # First chunk is small so jj=0 can start ASAP. Early chunks go on
            # the sync queue (the scalar queue is blocked ~1.3us by
            # ACT_TABLE_LOAD first); the tail chunk on scalar.
            CH = [(0, 1152), (1152, 2176)]
            nc.sync.dma_start(out=wt, in_=wt_d)
            for c0, c1 in CH:
                nc.sync.dma_start(out=rxt[:, 0:1, c0:c1], in_=rx_d[:, 0:1, c0:c1])
                nc.sync.dma_start(out=rxt[:, 1:2, c0:c1], in_=rx_d[:, 1:2, c0:c1])
            c0 = 2176
            nc.scalar.dma_start(out=rxt[:, 0:1, c0:], in_=rx_d[:, 0:1, c0:])
            nc.scalar.dma_start(out=rxt[:, 1:2, c0:], in_=rx_d[:, 1:2, c0:])

            # ---- PE warm-up during DMA wait (HAM un-throttle). FD=32 MMs
            # pipeline at ~27ns each; ~110 of them keep the PE busy ~3us so
            # the HAM SHORT window flips to 2.4 GHz as the first data lands.
            wps = psum_s.tile([P, 256], f32, tag="ps_s")
            for _ in range(WARMUP):
                nc.tensor.matmul(wps[:32, 0:32], wu[:, 0:32], wu[:, 0:32],
                                 start=True, stop=True)

            bf16 = mybir.dt.bfloat16

            def consume(t, ps, eng):
                fd = ps.shape[-1]
                if eng == "a":
                    sa = scr_a.tile([P, 1024], bf16, tag="sa")
                    nc.scalar.activation(sa[:, :fd], ps, relu, bias=zbias,
                                         scale=-1.0, accum_out=negp[:, t:t + 1])
                else:
                    sv = scr_v.tile([P, 1024], bf16, tag="sv")
                    nc.vector.tensor_scalar(sv[:, :fd], ps, 0.0, None, alu_min,
                                            op1=alu_add,
                                            accum_out=negp[:, t:t + 1])

            # ---- main loop: per row-block, 9 same-weight DoubleRow matmuls.
            # S (d0+d32) goes LAST: it needs late columns (b+4096..), and the
            # PE queue is FIFO — issuing it first would gate jj=0 on nearly
            # the whole transfer.
            for jj in range(RB):
                b = jj * P
                u = UNITS_PER_JJ * jj
                wsl = wt[:, :, jj * P:(jj + 1) * P]

                for m in range(4):
                    c0 = b + 128 + m * 1024
                    fd = 1024 if m < 3 else 896
                    mt = psum.tile([P, 1024], f32, tag="ps")
                    nc.tensor.matmul(mt[:, 0:512], wsl, rxt[:, :, c0:c0 + 512],
                                     start=True, stop=True, perf_mode=DR)
                    nc.tensor.matmul(mt[:, 512:fd], wsl,
                                     rxt[:, :, c0 + 512:c0 + fd],
                                     start=True, stop=True, perf_mode=DR)
                    consume(u + m, mt[:, :fd], "a" if m % 2 == 0 else "v")

                ms = psum_s.tile([P, 256], f32, tag="ps_s")
                nc.tensor.matmul(ms[:, 0:P], wsl, rxt[:, :, b:b + P],
                                 start=True, stop=True, perf_mode=DR)
                nc.tensor.matmul(ms[:, P:256], wsl, rxt[:, :, b + 4096:b + 4224],
                                 start=True, stop=True, perf_mode=DR)
                consume(u + 4, ms, "v")

            nc.sync.dma_start(out=neg_out, in_=negp)

    nc.compile()
    return nc


def _prep_inputs(x: np.ndarray, y: np.ndarray):
    """Host-side shard prep. O(N*D) only."""
    import ml_dtypes
    f8 = ml_dtypes.float8_e4m3fn

    x = np.ascontiguousarray(np.asarray(x, dtype=np.float32))
    y = np.asarray(y).astype(np.int64)
    assert x.shape == (N, D) and y.shape == (N,)

    # fp8-round x; derive sq from the ROUNDED x so device distance geometry
    # is self-consistent (diag lands at ~0, covered by +BIG anyway).
    x8 = x.astype(f8)
    xf = x8.astype(np.float32)
    sq = (xf * xf).sum(axis=1, dtype=np.float32)          # [N]
    oh = np.zeros((C, N), dtype=np.float32)
    oh[y, np.arange(N)] = 1.0

    xT8 = np.ascontiguousarray(x8.T)                      # [128, N] fp8

    def hi_lo(v):
        hi = v.astype(f8).astype(np.float32)
        lo = v - hi
        return hi, lo

    sq_hi, sq_lo = hi_lo(sq)
    sm1_hi, sm1_lo = hi_lo(sq - 1.0)

    # rhs aug rows: BIG*onehot ; sq_j hi/lo (lhs=1) ; ones (lhs=sm1 hi/lo)
    aug_r = np.empty((AUGK, N), dtype=np.float32)
    aug_r[:C] = BIG * oh
    aug_r[C] = sq_hi
    aug_r[C + 1] = sq_lo
    aug_r[C + 2] = 1.0
    aug_r[C + 3] = 1.0
    aug_r = aug_r.astype(f8)

    aug_l = np.empty((AUGK, N), dtype=np.float32)
    aug_l[:C] = oh
    aug_l[C] = 1.0
    aug_l[C + 1] = 1.0
    aug_l[C + 2] = sm1_hi
    aug_l[C + 3] = sm1_lo
    aug_l = aug_l.astype(f8)

    # weights: [128, 2, 1024] per core; x half = -2 x^T (exact in fp8),
    # aug half = aug_l zero-padded to 128 rows.
    m2xT = (-2.0 * xf.T).astype(f8)                       # [128, N], exact

    in_maps = []
    for k in range(NCORES):
        r0 = k * ROWS_PER_CORE
        idx = (r0 + np.arange(LOCAL_COLS)) % N
        rows = slice(r0, r0 + ROWS_PER_CORE)

        rx = np.zeros((P, 2, LOCAL_COLS), dtype=f8)
        rx[:, 0, :] = xT8[:, idx]
        rx[:AUGK, 1, :] = aug_r[:, idx]

        wt = np.zeros((P, 2, ROWS_PER_CORE), dtype=f8)
        wt[:, 0, :] = m2xT[:, rows]
        wt[:AUGK, 1, :] = aug_l[:, rows]

        in_maps.append({"rx": rx, "wt": wt})

    cnt = np.bincount(y, minlength=C).astype(np.float64)
    sum_sq_cnt = float((cnt * cnt).sum())
    pos_cnt = sum_sq_cnt - N
    neg_cnt = float(N) * N - sum_sq_cnt

    # pos term via the O(N*D) identity, f64 on the ORIGINAL f32 x:
    #   sum_{eq pairs} (sq_i + sq_j - 2 x_i.x_j)
    #     = 2 sum_i sq_i*cnt[y_i] - 2 sum_c ||sum_{i in c} x_i||^2
    x64 = x.astype(np.float64)
    sq64 = (x64 * x64).sum(axis=1)
    S = np.zeros((C, D), dtype=np.float64)
    np.add.at(S, y, x64)
    pos_sum = 2.0 * float((sq64 * cnt[y]).sum()) - 2.0 * float((S * S).sum())
    return in_maps, pos_cnt, neg_cnt, pos_sum


def _reduce_outputs(results):
    neg_sum = 0.0
    for r in results:
        neg_sum += float((r["neg_out"].astype(np.float64).sum(axis=0)
                          * UNIT_W * UNIT_SIGN).sum())
    return neg_sum


def kernel(x: np.ndarray, y: np.ndarray) -> np.ndarray:
    in_maps, pos_cnt, neg_cnt, pos_sum = _prep_inputs(x, y)

    if "nc" not in _cache:
        _cache["nc"] = _build_bass()
    nc = _cache["nc"]

    res = run_bass_kernel_spmd(nc, in_maps, core_ids=list(range(NCORES)),
                               trace=TRACE)
    _cache["last_results"] = res

    neg_sum = _reduce_outputs(res.results)
    loss = (pos_sum / pos_cnt + neg_sum / neg_cnt) / 2.0
    return np.float32(loss)
